# revision 28
# baseline (speedup 1.0000x reference)
"""Trainium2 Bass kernel for the ELGCA block (dwconv3x3+gelu || conv1x1+gelu
-> pooled linear attention), data-parallel over batch on 8 NeuronCores.

Self-contained: hardcodes shapes B=16, C=128, H=W=128, f32.
kernel(**inputs) takes full unsharded inputs, returns full output.

Per-core layout (BPC=2 local images b0,b1), partitions p=(b*64+c):
  x1 path: depthwise 3x3 runs on the PE as 9 diagonal-matmul accumulations
           into PSUM (diag(w_tap) preserves channels; rhs = shifted bf16 view
           of a zero-padded strip, 1 cyc/row).
  conv1x1: block-diagonal matmuls. A (q,k feeds softmax over ~1e3-scale
           logits) stays f32; B (v,l) runs bf16.
           A psum = [q(b0)|q(b1)|k(b0)|k(b1)], B = [v(b0)|v(b1)|l(b0)|l(b1)].
  out2:    one block-diagonal E matmul per 512-col chunk covers both images.
"""

import numpy as np
from contextlib import ExitStack

import concourse.bass as bass
import concourse.tile as tile
from concourse import bacc, mybir
from concourse import bass_utils
from concourse.masks import make_identity

F32 = mybir.dt.float32
F32R = mybir.dt.float32r
BF16 = mybir.dt.bfloat16
AX = mybir.AxisListType
ALU = mybir.AluOpType
ACTF = mybir.ActivationFunctionType

N_CORES = 8
B_TOT, C, H, W = 16, 128, 128, 128
BPC = B_TOT // N_CORES          # 2 images per core
HW = H * W                      # 16384
C2 = C // 2                     # 64
C4 = C // 4                     # 32
WP = W + 2                      # padded row width for dwconv strips
R = 16                          # dwconv row-strip height
NSTRIP = H // R                 # 8
NCH = 512                       # 512-col processing chunk (4 image rows)
NCHUNKS = HW // NCH             # 32
RPC = NCH // W                  # image rows per chunk (4)
NP = (H // 2) * (W // 2)        # 4096 pooled positions
W2 = W // 2

# dwconv taps in row-major (dy, dx) order
TAPS = [(dy, dx) for dy in (-1, 0, 1) for dx in (-1, 0, 1)]


def build_nc(loops=1):
    nc = bacc.Bacc("TRN2", target_bir_lowering=False, debug=False,
                   num_devices=N_CORES)
    x = nc.dram_tensor("x", [BPC, C, H, W], F32, kind="ExternalInput").ap()
    # host-precomputed weight layouts (see kernel() below)
    dgw_d = nc.dram_tensor("dgw", [128, 9 * 128], BF16,
                           kind="ExternalInput").ap()
    lhsAh_d = nc.dram_tensor("lhsAh", [128, 128], BF16,
                             kind="ExternalInput").ap()
    lhsAl_d = nc.dram_tensor("lhsAl", [128, 128], BF16,
                             kind="ExternalInput").ap()
    lhsB_d = nc.dram_tensor("lhsB", [128, 128], BF16,
                            kind="ExternalInput").ap()
    biases_d = nc.dram_tensor("biases", [128, 3], F32,
                              kind="ExternalInput").ap()
    out = nc.dram_tensor("out", [BPC, C, H, W], F32, kind="ExternalOutput").ap()

    x1v = x[:, 0:C2, :, :]                                  # [2, 64, H, W]
    x2v = x[:, C2:C, :, :].rearrange("b c h w -> b c (h w)")  # [2, 64, HW]
    ov1 = out[:, 0:C2, :, :].rearrange("b c h w -> b c (h w)")
    ovl = out[:, C2:96, :, :].rearrange("b c h w -> b c (h w)")
    ov2 = out[:, 96:128, :, :].rearrange("b c h w -> b c (h w)")

    with tile.TileContext(nc) as tc, ExitStack() as ctx:
        consts = ctx.enter_context(tc.tile_pool(name="consts", bufs=1))
        xinp = ctx.enter_context(tc.tile_pool(name="xinp", bufs=2))
        rhsp = ctx.enter_context(tc.tile_pool(name="rhsp", bufs=2))
        qgp = ctx.enter_context(tc.tile_pool(name="qgp", bufs=2))
        packp = ctx.enter_context(tc.tile_pool(name="packp", bufs=1))
        poolt = ctx.enter_context(tc.tile_pool(name="poolt", bufs=1))
        attnp = ctx.enter_context(tc.tile_pool(name="attnp", bufs=1))
        stgp = ctx.enter_context(tc.tile_pool(name="stgp", bufs=2))
        ps_dw = ctx.enter_context(
            tc.tile_pool(name="ps_dw", bufs=2, space="PSUM"))
        ps_conv = ctx.enter_context(
            tc.tile_pool(name="ps_conv", bufs=3, space="PSUM"))
        ps_sm = ctx.enter_context(
            tc.tile_pool(name="ps_sm", bufs=2, space="PSUM"))
        ps_tr = ctx.enter_context(
            tc.tile_pool(name="ps_tr", bufs=1, space="PSUM"))

        # ------- constants (all layouts precomputed on host) -------
        id_f32 = consts.tile([128, 128], F32)
        make_identity(nc, id_f32[:])

        dgw = consts.tile([128, 9 * 128], BF16)
        dgw3 = dgw.rearrange("p (t q) -> p t q", q=128)
        lhsAh = consts.tile([128, 128], BF16)
        lhsAl = consts.tile([128, 128], BF16)
        lhsB = consts.tile([128, 128], BF16)
        biases_t = consts.tile([128, 3], F32)
        dwb_t = biases_t[:, 0:1]
        biasA = biases_t[:, 1:2]
        biasB = biases_t[:, 2:3]

        def one_pass():
            # v_pack (bf16): [v(b0) 0:32 | v(b1) 32:64]
            v_pack = packp.tile([C2, HW], BF16, tag="v_pack")
            # rt (f32): h-pooled [q(b0)|q(b1) sums 0:64 | k(b0)|k(b1) max 64:128]
            rt = packp.tile([128, H * W2], F32, tag="rt")
            # qf[0:64] = [qf(b0)|qf(b1)]; kf[64:128] = [kf(b0)|kf(b1) max],
            # kf[32:64] = copy of kf(b1) at a legal matmul base partition
            qf = poolt.tile([C2, NP], F32, tag="qf")
            kf = poolt.tile([128, NP], F32, tag="kf")
            qkts_ps = [ps_sm.tile([C4, C4], F32, tag="sm", name=f"qkt{b}")
                       for b in range(BPC)]
            E_blk = attnp.tile([C2, C2], BF16, tag="E_blk")
            nc.vector.memset(E_blk[:], 0.0)

            xin_tiles = {}

            def emit_strip_load(g):
                # image rows 16g-1 .. 16g+16 (incl. dw halo) into 18 padded
                # rows of [128, 18, 130]; borders memset to zero.
                xin = xinp.tile([128, (R + 2) * WP], F32, tag="xin",
                                name=f"xin{g}")
                xin3 = xin.rearrange("p (r w) -> p r w", w=WP)
                nc.gpsimd.memset(xin3[:, :, 0:1], 0.0)
                nc.gpsimd.memset(xin3[:, :, WP - 1:WP], 0.0)
                y0 = g * R
                ys = max(y0 - 1, 0)
                ye = min(y0 + R + 1, H)
                rs = 0 if g > 0 else 1
                if g == 0:
                    nc.gpsimd.memset(xin3[:, 0:1, :], 0.0)
                if g == NSTRIP - 1:
                    nc.gpsimd.memset(xin3[:, R + 1:R + 2, :], 0.0)
                nc.sync.dma_start(xin3[0:C2, rs:rs + (ye - ys), 1:W + 1],
                                  x1v[0, :, ys:ye, :])
                nc.sync.dma_start(xin3[C2:128, rs:rs + (ye - ys), 1:W + 1],
                                  x1v[1, :, ys:ye, :])
                xb = xinp.tile([128, (R + 2) * WP], BF16, tag="xb",
                               name=f"xb{g}")
                nc.vector.tensor_copy(xb[:], xin[:])
                xin_tiles[g] = xb.rearrange("p (r w) -> p r w", w=WP)

            def emit_dw_chunk(j):
                # depthwise 3x3 + gelu for image rows 4j..4j+3 on the PE
                g, k = j // RPC, j % RPC
                xin3 = xin_tiles[g]
                dwp = ps_dw.tile([128, NCH], F32, tag="dw")
                for ti, (dy, dx) in enumerate(TAPS):
                    rhs = xin3[:, 4 * k + dy + 1:4 * k + dy + 5,
                               1 + dx:1 + dx + W]
                    nc.tensor.matmul(dwp[:], dgw3[:, ti, :], rhs,
                                     start=(ti == 0), stop=(ti == 8))
                if k == 0:
                    stx1 = stgp.tile([128, RPC * NCH], F32, tag="stx1",
                                     name=f"stx1_{g}")
                    xin_tiles[g + 100] = stx1
                else:
                    stx1 = xin_tiles[g + 100]
                nc.scalar.activation(stx1[:, k * NCH:(k + 1) * NCH], dwp[:],
                                     ACTF.Gelu, bias=dwb_t)
                if k == RPC - 1:
                    cols = bass.ts(g, RPC * NCH)
                    nc.scalar.dma_start(ov1[0, :, cols], stx1[0:C2, :])
                    nc.scalar.dma_start(ov1[1, :, cols], stx1[C2:128, :])
                    if g + 2 < NSTRIP:
                        emit_strip_load(g + 2)

            def emit_attn_pools(q):
                # vertical pool for output rows oy in [16q, 16q+16)
                oy0, oy1 = q * 16, (q + 1) * 16
                rq = rt[0:C2, :].rearrange(
                    "p (h2 two w2) -> p h2 two w2", two=2, w2=W2)
                qf3 = qf.rearrange("p (h2 w2) -> p h2 w2", w2=W2)
                nc.gpsimd.tensor_add(qf3[:, oy0:oy1, :],
                                     rq[:, oy0:oy1, 0, :],
                                     rq[:, oy0:oy1, 1, :])
                lo = max(oy0, 1)
                nc.gpsimd.tensor_add(qf3[:, lo:oy1, :], qf3[:, lo:oy1, :],
                                     rq[:, lo - 1:oy1 - 1, 1, :])
                rk = rt[C2:128, :].rearrange(
                    "p (h2 two w2) -> p h2 two w2", two=2, w2=W2)
                kf3 = kf[C2:128, :].rearrange("p (h2 w2) -> p h2 w2", w2=W2)
                nc.vector.tensor_max(kf3[:, oy0:oy1, :],
                                     rk[:, oy0:oy1, 0, :],
                                     rk[:, oy0:oy1, 1, :])
                # k(b1) at base 96 (illegal matmul base) -> kf[32:64]
                hcols = bass.ts(q, 16 * W2)
                nc.sync.dma_start(kf[C4:C2, hcols], kf[96:128, hcols])

            def emit_attn_trqk(q):
                # transposes + qk accumulation for this quarter's 8 chunks
                for bi in range(BPC):
                    Pq = C4 * bi
                    Pk = C2 if bi == 0 else C4
                    tts = []
                    for (srcT, Ps, tg) in ((qf, Pq, "qT"), (kf, Pk, "kT")):
                        ps = ps_tr.tile([128, 256], F32, tag="trps")
                        for jj in range(8):
                            j2 = q * 8 + jj
                            nc.tensor.transpose(
                                ps[:, jj * C4:(jj + 1) * C4],
                                srcT[Ps:Ps + C4, j2 * 128:(j2 + 1) * 128],
                                id_f32[Ps:Ps + C4, Ps:Ps + C4])
                        tT = attnp.tile([128, 256], F32, tag=tg, bufs=2,
                                        name=f"{tg}{bi}")
                        nc.vector.tensor_copy(tT[:], ps[:])
                        tts.append(tT)
                    qT, kT = tts
                    for jj in range(8):
                        nc.tensor.matmul(
                            qkts_ps[bi][:],
                            kT[:, jj * C4:(jj + 1) * C4],
                            qT[:, jj * C4:(jj + 1) * C4],
                            start=(q == 0 and jj == 0),
                            stop=(q == 3 and jj == 7))

            # ---------- main chunk loop: conv1x1 + dwconv interleaved ------
            rhs = None
            rhs_next = rhsp.tile([128, RPC * NCH], F32, tag="rhs", name="rhs0")
            nc.sync.dma_start(lhsAh[:], lhsAh_d)
            nc.sync.dma_start(lhsAl[:], lhsAl_d)
            nc.sync.dma_start(lhsB[:], lhsB_d)
            nc.sync.dma_start(biases_t[:], biases_d)
            nc.sync.dma_start(rhs_next[0:C2, :], x2v[0, :, 0:RPC * NCH])
            nc.sync.dma_start(rhs_next[C2:128, :], x2v[1, :, 0:RPC * NCH])
            nc.sync.dma_start(dgw[:], dgw_d)
            emit_strip_load(0)
            emit_strip_load(1)
            lst = None
            for j in range(NCHUNKS):
                g, k = j // RPC, j % RPC
                cols = bass.ts(j, NCH)
                if k == 0:
                    rhs = rhs_next
                    if g + 1 < NSTRIP:
                        rhs_next = rhsp.tile([128, RPC * NCH], F32, tag="rhs",
                                             name=f"rhs{g + 1}")
                        cols4 = bass.ts(g + 1, RPC * NCH)
                        nc.sync.dma_start(rhs_next[0:C2, :],
                                          x2v[0, :, cols4])
                        nc.sync.dma_start(rhs_next[C2:128, :],
                                          x2v[1, :, cols4])
                rhsv = rhs[:, k * NCH:(k + 1) * NCH]
                rhsb = rhsp.tile([128, NCH], BF16, tag="rhsb", bufs=3)
                nc.vector.tensor_copy(rhsb[:], rhsv)
                rhsl = rhsp.tile([128, NCH], BF16, tag="rhsl", bufs=3)
                nc.vector.tensor_sub(rhsl[:], rhsv, rhsb[:])

                Ap = ps_conv.tile([128, NCH], F32, tag="conv")
                nc.tensor.matmul(Ap[:], lhsAh[:], rhsb[:],
                                 start=True, stop=False)
                nc.tensor.matmul(Ap[:], lhsAl[:], rhsb[:],
                                 start=False, stop=False)
                nc.tensor.matmul(Ap[:], lhsAh[:], rhsl[:],
                                 start=False, stop=True)
                qg = qgp.tile([128, NCH], F32, tag="qg", bufs=3)
                nc.scalar.activation(qg[:], Ap[:], ACTF.Gelu,
                                     bias=biasA)

                # fused horizontal pooling into rt (both batches per op)
                rrows = rt[:, j * RPC * W2:(j + 1) * RPC * W2]
                Xq = qg[0:C2, :].rearrange(
                    "p (h w2 two) -> p h w2 two", h=RPC, two=2)
                r3 = rrows[0:C2, :].rearrange("p (h w2) -> p h w2", h=RPC)
                nc.gpsimd.tensor_add(r3[:], Xq[:, :, :, 0], Xq[:, :, :, 1])
                nc.gpsimd.tensor_add(r3[:, :, 1:W2], r3[:, :, 1:W2],
                                     Xq[:, :, 0:W2 - 1, 1])
                Xk = qg[C2:128, :].rearrange(
                    "p (h w2 two) -> p h w2 two", h=RPC, two=2)
                m3 = rrows[C2:128, :].rearrange("p (h w2) -> p h w2", h=RPC)
                nc.vector.tensor_max(m3[:], Xk[:, :, :, 0], Xk[:, :, :, 1])

                Bp = ps_conv.tile([128, NCH], F32, tag="conv")
                nc.tensor.matmul(Bp[:], lhsB[:], rhsb[:],
                                 start=True, stop=True)
                if j % 2 == 0:
                    lst = stgp.tile([128, 2 * NCH], F32, tag="lst", bufs=3,
                                    name=f"lst{j // 2}")
                lcols = slice((j % 2) * NCH, (j % 2 + 1) * NCH)
                nc.scalar.activation(lst[:, lcols], Bp[:], ACTF.Gelu,
                                     bias=biasB)
                nc.vector.tensor_copy(v_pack[:, cols], lst[0:C2, lcols])
                if j % 2 == 1:
                    cols2 = bass.ts(j // 2, 2 * NCH)
                    nc.scalar.dma_start(ovl[:, :, cols2], lst[C2:128, :])

                if j >= 4:
                    emit_dw_chunk(j - 4)
                if j % 8 == 7:
                    emit_attn_pools(j // 8)
                if j >= 10 and j % 8 == 2:
                    emit_attn_trqk((j - 10) // 8)

            # ---------- remaining dwconv strips (all Gelu work before the
            # Exp act-table switch), then attention stats + blockdiag E ----
            emit_dw_chunk(NCHUNKS - 4)
            emit_dw_chunk(NCHUNKS - 3)
            emit_attn_trqk(3)
            emit_dw_chunk(NCHUNKS - 2)
            emit_dw_chunk(NCHUNKS - 1)
            for bi in range(BPC):
                qkts = attnp.tile([C4, C4], F32, tag="qkts")
                nc.scalar.mul(qkts[:], qkts_ps[bi][:], 1.0 / 9.0)
                nmax = attnp.tile([C4, 1], F32, tag="nmax")
                nc.vector.tensor_reduce(nmax[:], qkts[:], axis=AX.X,
                                        op=ALU.max, negate=True)
                ET = attnp.tile([C4, C4], F32, tag="ET")
                nc.scalar.activation(ET[:], qkts[:], ACTF.Exp,
                                     bias=nmax[:, 0:1])
                ssum = attnp.tile([C4, 1], F32, tag="ssum")
                nc.vector.reduce_sum(ssum[:], ET[:], axis=AX.X)
                rec = attnp.tile([C4, 1], F32, tag="rec")
                nc.vector.reciprocal(rec[:], ssum[:])
                ETn = attnp.tile([C4, C4], F32, tag="ETn")
                nc.vector.tensor_scalar_mul(ETn[:], ET[:], rec[:, 0:1])
                etp = ps_sm.tile([C4, C4], F32, tag="sm", name=f"etp{bi}")
                nc.tensor.transpose(etp[:], ETn[:], id_f32[0:C4, 0:C4])
                nc.vector.tensor_copy(E_blk[C4 * bi:C4 * bi + C4,
                                            C4 * bi:C4 * bi + C4], etp[:])

            # ---------- out2: one blockdiag matmul per chunk ----------
            sto2 = None
            for j in range(NCHUNKS):
                g, k = j // RPC, j % RPC
                cols = bass.ts(j, NCH)
                o2pool = ps_dw if j % 2 == 0 else ps_conv
                o2tag = "dw" if j % 2 == 0 else "conv"
                o2 = o2pool.tile([128, NCH], F32, tag=o2tag, name=f"o2_{j}")
                nc.tensor.matmul(o2[0:C2, :], E_blk[:], v_pack[:, cols],
                                 start=True, stop=True)
                if j % 2 == 0:
                    sto2 = stgp.tile([C2, 2 * NCH], F32, tag="sto2", bufs=3,
                                     name=f"sto2_{j // 2}")
                    nc.vector.tensor_copy(sto2[:, 0:NCH], o2[0:C2, :])
                else:
                    nc.scalar.copy(sto2[:, NCH:2 * NCH], o2[0:C2, :])
                    cols2 = bass.ts(j // 2, 2 * NCH)
                    nc.sync.dma_start(ov2[:, :, cols2], sto2[:, :])

        for _ in range(loops):
            one_pass()

    nc.compile()
    return nc


_NC_CACHE = None


def _get_nc():
    global _NC_CACHE
    if _NC_CACHE is None:
        _NC_CACHE = build_nc()
    return _NC_CACHE


def host_consts(dw_w, dw_b, qkvl_w, qkvl_b):
    """Precompute the on-chip weight layouts (diagonal dw taps, block-diagonal
    conv1x1 matrices, packed bias columns)."""
    dw_b = np.asarray(dw_b, np.float32)
    w9 = np.asarray(dw_w, np.float32).reshape(C2, 9)
    Wq = np.asarray(qkvl_w, np.float32).reshape(C, C2)
    qb = np.asarray(qkvl_b, np.float32)

    dgw = np.zeros((128, 9, 128), np.float32)
    pp = np.arange(128)
    for t in range(9):
        dgw[pp, t, pp] = w9[pp % C2, t]
    lhsA = np.zeros((128, 128), np.float32)
    lhsB = np.zeros((128, 128), np.float32)
    lhsA[0:C2, 0:C4] = Wq[0:C4].T
    lhsA[C2:128, C4:C2] = Wq[0:C4].T
    lhsA[0:C2, C2:96] = Wq[C4:C2].T
    lhsA[C2:128, 96:128] = Wq[C4:C2].T
    lhsB[0:C2, 0:C4] = Wq[C2:96].T
    lhsB[C2:128, C4:C2] = Wq[C2:96].T
    lhsB[0:C2, C2:96] = Wq[96:128].T
    lhsB[C2:128, 96:128] = Wq[96:128].T
    biases = np.zeros((128, 3), np.float32)
    biases[:, 0] = np.concatenate([dw_b, dw_b])
    biases[:, 1] = np.concatenate([qb[0:C4], qb[0:C4], qb[C4:C2], qb[C4:C2]])
    biases[:, 2] = np.concatenate([qb[C2:96], qb[C2:96],
                                   qb[96:128], qb[96:128]])
    import ml_dtypes
    lhsAh = lhsA.astype(ml_dtypes.bfloat16)
    lhsAl = (lhsA - lhsAh.astype(np.float32)).astype(ml_dtypes.bfloat16)
    return {
        "dgw": np.ascontiguousarray(
            dgw.reshape(128, 9 * 128).astype(ml_dtypes.bfloat16)),
        "lhsAh": lhsAh,
        "lhsAl": lhsAl,
        "lhsB": lhsB.astype(ml_dtypes.bfloat16),
        "biases": biases,
    }


def kernel(x, dw_w, dw_b, qkvl_w, qkvl_b):
    x = np.ascontiguousarray(np.asarray(x, dtype=np.float32))
    shared = host_consts(dw_w, dw_b, qkvl_w, qkvl_b)
    nc = _get_nc()
    in_maps = [
        {"x": x[c * BPC:(c + 1) * BPC], **shared} for c in range(N_CORES)
    ]
    res = bass_utils.run_bass_kernel_spmd(nc, in_maps,
                                          core_ids=list(range(N_CORES)))
    return np.concatenate([res.results[c]["out"] for c in range(N_CORES)],
                          axis=0)
